# revision 18
# baseline (speedup 1.0000x reference)
"""FFTConv2d kernel for trn2, 8 NeuronCores.

Math: reference einsum 'bchw,oihw->bohw' factorizes:
  Y[b,o] = conv_same(sum_c x[b,c], flip(sum_i w[o,i])) + bias[o]
i.e. a single-channel 3x3 "same" convolution per (b,o) pair.

Per core (2 batches), all SBUF data fp16 (PSUM accum fp32):
  1. xin [128 (b,c), 16384] <- x fp16, 9 HBM DMA pieces (SP ring).
  2. Channel-sum: ones-indicator matmul pairs -> PSUM [2, 2x512];
     one FD=1024 copy (DVE/Act alternating) per pair drains 8 image
     rows into the padded staging = P9 partitions {0,1} (row stride
     130, zero borders memset once).
  3. P9 [19, 16902], partition 2m+b holds staging shifted by
     jj*130+i' (m=3i'+jj); m=0 IS the staging; m=1..8 built by 2-D
     self-copy DMAs in 2 row-bands (8 DMAs each, Pool/SP).
     Partition 18 = ones (bias rides the conv matmul).
  4. Conv: 33 flat 512-col chunks (yt rows are 130 wide, 2 junk cols
     stripped on host); K=19 fp16 matmul pairs into the same 4-deep
     2-bank PSUM rotation; FD=1024 contiguous copies -> yt.
  5. yt [128, 16640] -> HBM in 8 pieces on the Pool (SWDGE) ring,
     fp16; host strips junk cols + upconverts to fp32.

PSUM: four [128, 1024] fp32 tensors (2 banks each = all 8 banks),
time-shared: cs pairs use partitions 0:2, conv pairs all 128.
"""

import os
import sys
from functools import lru_cache

import numpy as np

for _p in ("/opt/trn_rl_repo", "/root/.axon_site/_ro/trn_rl_repo"):
    if os.path.isdir(_p) and _p not in sys.path:
        sys.path.insert(0, _p)

B, CIN, COUT, H, W = 16, 64, 64, 128, 128
N_CORES = 8
BPC = B // N_CORES  # 2
NPART = BPC * CIN  # 128
NOUT = BPC * COUT  # 128
WROW = W + 2  # 130
HW = H * W  # 16384
HHW = H * WROW  # 16640 (130-wide output rows)
LSP = (H + 2) * WROW + 2  # 16902 (padded staging length)
NK = BPC * 9 + 1  # 19
NCV = 33  # conv chunks: 32x512 + 1x256


@lru_cache(maxsize=1)
def _build():
    import concourse.bacc as bacc
    import concourse.mybir as mybir
    import concourse.tile as tile
    from concourse.ap import AP

    f32 = mybir.dt.float32
    f16 = mybir.dt.float16

    nc = bacc.Bacc("TRN2", target_bir_lowering=False, debug=False, num_devices=N_CORES)

    xh = nc.dram_tensor("xh", [NPART, HW], f16, kind="ExternalInput")
    wbh = nc.dram_tensor("wb", [NK, NOUT], f16, kind="ExternalInput")
    onesp = nc.dram_tensor("ones_p", [1, HHW], f16, kind="ExternalInput")
    y = nc.dram_tensor("y", [NOUT, HHW], f16, kind="ExternalOutput")
    dump = os.environ.get("KDUMP")
    if dump:
        p9_d = nc.dram_tensor("p9_d", [NK, HHW], f16, kind="ExternalOutput")

    with tile.TileContext(nc) as tc:
        with (
            tc.tile_pool(name="main", bufs=1) as mp,
            tc.tile_pool(name="ps", bufs=1, space="PSUM") as ps_pool,
        ):
            xin = mp.tile([NPART, HW], f16, tag="xin")
            p9 = mp.tile([NK, LSP], f16, tag="p9")
            yt = mp.tile([NOUT, HHW], f16, tag="yt")
            ones_t = mp.tile([NPART, BPC], f16, tag="ones_t")
            wb_t = mp.tile([NK, NOUT], f16, tag="wb")
            fence = mp.tile([1, 64], f16, tag="fence")

            p9t = p9.tensor

            csb = [
                ps_pool.tile([BPC, 512], f32, tag=f"cs{i}", name=f"cs{i}")
                for i in range(4)
            ]
            cvb = [
                ps_pool.tile([NOUT, 512], f32, tag=f"cv{i}", name=f"cv{i}")
                for i in range(4)
            ]

            # constants on the idle Pool (SWDGE) ring so the Act queue
            # starts clean (a late Act start skews the static schedule)
            nc.gpsimd.dma_start(out=wb_t[:, :], in_=wbh.ap()[:, :])
            nc.gpsimd.dma_start(out=p9[NK - 1 : NK, 0:HHW], in_=onesp.ap()[0:1, :])

            # ones indicator [128, 2]: col b is 1 for partitions of batch b
            nc.vector.memset(ones_t[0:CIN, 0:1], 1.0)
            nc.vector.memset(ones_t[0:CIN, 1:2], 0.0)
            nc.vector.memset(ones_t[CIN:NPART, 0:1], 0.0)
            nc.vector.memset(ones_t[CIN:NPART, 1:2], 1.0)

            # staging zero borders in P9 partitions {0, 1}:
            # row -1, row 128 + tail, and (right col, next left col) pairs
            nc.vector.memset(
                AP(tensor=p9t, offset=0, ap=[[LSP, BPC], [1, WROW]]), 0.0
            )
            nc.vector.memset(
                AP(
                    tensor=p9t,
                    offset=(H + 1) * WROW,
                    ap=[[LSP, BPC], [1, LSP - (H + 1) * WROW]],
                ),
                0.0,
            )
            nc.vector.memset(
                AP(
                    tensor=p9t,
                    offset=WROW - 1,
                    ap=[[LSP, BPC], [WROW, H + 1], [1, 2]],
                ),
                0.0,
            )

            # input: 9 pieces on the SP ring
            pieces = [(0, 1024), (1024, 1024)] + [
                (2048 * q, 2048) for q in range(1, 8)
            ]
            for o, n in pieces:
                nc.sync.dma_start(out=xin[:, o : o + n], in_=xh.ap()[:, o : o + n])

            copy_engines = [nc.vector, nc.scalar]

            def ecopy(idx, dst, src):
                eng = copy_engines[idx % 2]
                if eng is nc.vector:
                    eng.tensor_copy(dst, src)
                else:
                    eng.copy(dst, src)

            def emit_cs(k):
                # ones-matmul of 512 cols (4 rows) -> [2, 512]; 1 copy
                ps = csb[k % 4]
                pst = ps.tensor
                nc.tensor.matmul(
                    ps[:, :],
                    ones_t[:, :],
                    xin[:, 512 * k : 512 * k + 512],
                    start=True,
                    stop=True,
                )
                dst = AP(
                    tensor=p9t,
                    offset=(4 * k + 1) * WROW + 1,
                    ap=[[LSP, BPC], [WROW, 4], [1, W]],
                )
                src = AP(
                    tensor=pst, offset=0, ap=[[512, BPC], [W, 4], [1, W]]
                )
                ecopy(k, dst, src)

            def emit_band(r0, r1, engines):
                # P9 partitions m=1..8 <- shifted copies of partitions {0,1}
                ln = (r1 - r0) * WROW
                n = 0
                for m in range(1, 9):
                    ip, jj = divmod(m, 3)
                    eng = engines[n % len(engines)]
                    n += 1
                    eng.dma_start(
                        out=AP(
                            tensor=p9t,
                            offset=2 * m * LSP + r0 * WROW,
                            ap=[[LSP, BPC], [1, ln]],
                        ),
                        in_=AP(
                            tensor=p9t,
                            offset=(r0 + jj) * WROW + ip,
                            ap=[[LSP, BPC], [1, ln]],
                        ),
                        single_packet=(eng is nc.gpsimd),
                    )

            out_engines = [nc.gpsimd, nc.sync]

            nfence = [0]

            def emit_fence(engines):
                # tiny DMA after a band on the same ring(s): gives the
                # scheduler an early-completing proxy for "band done"
                for eng in engines:
                    o = 16 * (nfence[0] % 4)
                    nfence[0] += 1
                    eng.dma_start(
                        out=fence[0:1, o : o + 16], in_=wb_t[0:1, 0:16]
                    )

            def emit_conv(j):
                cv = cvb[j % 4]
                nn = 512 if j < NCV - 1 else 256
                nc.tensor.matmul(
                    cv[:, :nn],
                    wb_t[:, :],
                    p9[:, 512 * j : 512 * j + nn],
                    start=True,
                    stop=True,
                )
                ecopy(j, yt[:, 512 * j : 512 * j + nn], cv[:, :nn])

            def emit_out(q):
                out_engines[q % 2].dma_start(
                    out=y.ap()[:, 2080 * q : 2080 * (q + 1)],
                    in_=yt[:, 2080 * q : 2080 * (q + 1)],
                )

            # out piece q (rows 16q..) ready after conv chunk: {chunk: piece}
            out_after = {4: 0, 8: 1, 12: 2, 16: 3, 20: 4, 24: 5, 28: 6, 32: 7}

            def emit_conv_full(j):
                emit_conv(j)
                if j in out_after:
                    emit_out(out_after[j])

            for k in range(32):
                emit_cs(k)
                if k == 12:
                    # band A (P9 flat [0, 6240)) needs staging rows <= 48
                    emit_band(0, 48, [nc.gpsimd])
                    emit_fence([nc.gpsimd])
            emit_band(48, H, [nc.gpsimd, nc.gpsimd, nc.sync])
            emit_fence([nc.gpsimd, nc.sync])
            for j in range(NCV):
                emit_conv_full(j)
            if dump:
                nc.sync.dma_start(out=p9_d.ap()[:, :], in_=p9[:, 0:HHW])

    nc.compile()
    return nc


def _host_prep(x, weight, bias):
    wsum = weight.sum(axis=1)  # [COUT, 3, 3]
    wb = np.zeros((NK, NOUT), np.float32)
    for b in range(BPC):
        for ip in range(3):
            for jj in range(3):
                wb[2 * (3 * ip + jj) + b, b * COUT : (b + 1) * COUT] = wsum[
                    :, 2 - jj, 2 - ip
                ]
    wb[NK - 1, :] = np.tile(bias, BPC)
    wb = wb.astype(np.float16)
    ones_p = np.ones((1, HHW), np.float16)

    in_maps = []
    for r in range(N_CORES):
        xhr = np.ascontiguousarray(
            x[r * BPC : (r + 1) * BPC].reshape(NPART, HW)
        ).astype(np.float16)
        in_maps.append({"xh": xhr, "wb": wb, "ones_p": ones_p})
    return in_maps


def kernel(x, weight, bias):
    from concourse.bass_utils import run_bass_kernel_spmd

    x = np.asarray(x)
    weight = np.asarray(weight)
    bias = np.asarray(bias)
    nc = _build()
    in_maps = _host_prep(x, weight, bias)
    res = run_bass_kernel_spmd(nc, in_maps, core_ids=list(range(N_CORES)))
    out = np.concatenate(
        [
            np.asarray(res.results[r]["y"])
            .astype(np.float32)
            .reshape(BPC, COUT, H, WROW)[:, :, :, :W]
            for r in range(N_CORES)
        ],
        axis=0,
    )
    return out


# revision 19
# speedup vs baseline: 1.0394x; 1.0394x over previous
"""FFTConv2d kernel for trn2, 8 NeuronCores.

Math: reference einsum 'bchw,oihw->bohw' factorizes:
  Y[b,o] = conv_same(sum_c x[b,c], flip(sum_i w[o,i])) + bias[o]
i.e. a single-channel 3x3 "same" convolution per (b,o) pair.

Per core (2 batches), all SBUF data fp16 (PSUM accum fp32):
  1. xin [128 (b,c), 16384] <- x fp16, 9 HBM DMA pieces (SP ring).
  2. Channel-sum: ones-indicator matmul pairs -> PSUM [2, 2x512];
     one FD=1024 copy (DVE/Act alternating) per pair drains 8 image
     rows into the padded staging = P9 partitions {0,1} (row stride
     130, zero borders memset once).
  3. P9 [19, 16902], partition 2m+b holds staging shifted by
     jj*130+i' (m=3i'+jj); m=0 IS the staging; m=1..8 built by 2-D
     self-copy DMAs in 2 row-bands (8 DMAs each, Pool/SP).
     Partition 18 = ones (bias rides the conv matmul).
  4. Conv: 33 flat 512-col chunks (yt rows are 130 wide, 2 junk cols
     stripped on host); K=19 fp16 matmul pairs into the same 4-deep
     2-bank PSUM rotation; FD=1024 contiguous copies -> yt.
  5. yt [128, 16640] -> HBM in 8 pieces on the Pool (SWDGE) ring,
     fp16; host strips junk cols + upconverts to fp32.

PSUM: four [128, 1024] fp32 tensors (2 banks each = all 8 banks),
time-shared: cs pairs use partitions 0:2, conv pairs all 128.
"""

import os
import sys
from functools import lru_cache

import numpy as np

for _p in ("/opt/trn_rl_repo", "/root/.axon_site/_ro/trn_rl_repo"):
    if os.path.isdir(_p) and _p not in sys.path:
        sys.path.insert(0, _p)

B, CIN, COUT, H, W = 16, 64, 64, 128, 128
N_CORES = 8
BPC = B // N_CORES  # 2
NPART = BPC * CIN  # 128
NOUT = BPC * COUT  # 128
WROW = W + 2  # 130
HW = H * W  # 16384
HHW = H * WROW  # 16640 (130-wide output rows)
LSP = (H + 2) * WROW + 2  # 16902 (padded staging length)
NK = BPC * 9 + 1  # 19
NCV = 33  # conv chunks: 32x512 + 1x256


@lru_cache(maxsize=1)
def _build():
    import concourse.bacc as bacc
    import concourse.mybir as mybir
    import concourse.tile as tile
    from concourse.ap import AP

    f32 = mybir.dt.float32
    f16 = mybir.dt.float16

    nc = bacc.Bacc("TRN2", target_bir_lowering=False, debug=False, num_devices=N_CORES)

    xh = nc.dram_tensor("xh", [NPART, HW], f16, kind="ExternalInput")
    wbh = nc.dram_tensor("wb", [NK, NOUT], f16, kind="ExternalInput")
    onesp = nc.dram_tensor("ones_p", [1, HHW], f16, kind="ExternalInput")
    y = nc.dram_tensor("y", [NOUT, HHW], f16, kind="ExternalOutput")
    dump = os.environ.get("KDUMP")
    if dump:
        p9_d = nc.dram_tensor("p9_d", [NK, HHW], f16, kind="ExternalOutput")

    with tile.TileContext(nc) as tc:
        with (
            tc.tile_pool(name="main", bufs=1) as mp,
            tc.tile_pool(name="ps", bufs=1, space="PSUM") as ps_pool,
        ):
            xin = mp.tile([NPART, HW], f16, tag="xin")
            p9 = mp.tile([NK, LSP], f16, tag="p9")
            yt = mp.tile([NOUT, HHW], f16, tag="yt")
            ones_t = mp.tile([NPART, BPC], f16, tag="ones_t")
            wb_t = mp.tile([NK, NOUT], f16, tag="wb")

            p9t = p9.tensor

            csb = [
                ps_pool.tile([BPC, 512], f32, tag=f"cs{i}", name=f"cs{i}")
                for i in range(4)
            ]
            cvb = [
                ps_pool.tile([NOUT, 512], f32, tag=f"cv{i}", name=f"cv{i}")
                for i in range(4)
            ]

            # constants on the idle Pool (SWDGE) ring so the Act queue
            # starts clean (a late Act start skews the static schedule)
            nc.gpsimd.dma_start(out=wb_t[:, :], in_=wbh.ap()[:, :])
            nc.gpsimd.dma_start(out=p9[NK - 1 : NK, 0:HHW], in_=onesp.ap()[0:1, :])

            # ones indicator [128, 2]: col b is 1 for partitions of batch b
            nc.vector.memset(ones_t[0:CIN, 0:1], 1.0)
            nc.vector.memset(ones_t[0:CIN, 1:2], 0.0)
            nc.vector.memset(ones_t[CIN:NPART, 0:1], 0.0)
            nc.vector.memset(ones_t[CIN:NPART, 1:2], 1.0)

            # staging zero borders in P9 partitions {0, 1}:
            # row -1, row 128 + tail, and (right col, next left col) pairs
            nc.vector.memset(
                AP(tensor=p9t, offset=0, ap=[[LSP, BPC], [1, WROW]]), 0.0
            )
            nc.vector.memset(
                AP(
                    tensor=p9t,
                    offset=(H + 1) * WROW,
                    ap=[[LSP, BPC], [1, LSP - (H + 1) * WROW]],
                ),
                0.0,
            )
            nc.vector.memset(
                AP(
                    tensor=p9t,
                    offset=WROW - 1,
                    ap=[[LSP, BPC], [WROW, H + 1], [1, 2]],
                ),
                0.0,
            )

            # input: 9 pieces on the SP ring
            pieces = [(0, 1024), (1024, 1024)] + [
                (2048 * q, 2048) for q in range(1, 8)
            ]
            for o, n in pieces:
                nc.sync.dma_start(out=xin[:, o : o + n], in_=xh.ap()[:, o : o + n])

            copy_engines = [nc.vector, nc.scalar]

            def ecopy(idx, dst, src):
                eng = copy_engines[idx % 2]
                if eng is nc.vector:
                    eng.tensor_copy(dst, src)
                else:
                    eng.copy(dst, src)

            def emit_cs(k):
                # ones-matmul of 512 cols (4 rows) -> [2, 512]; 1 copy
                ps = csb[k % 4]
                pst = ps.tensor
                nc.tensor.matmul(
                    ps[:, :],
                    ones_t[:, :],
                    xin[:, 512 * k : 512 * k + 512],
                    start=True,
                    stop=True,
                )
                dst = AP(
                    tensor=p9t,
                    offset=(4 * k + 1) * WROW + 1,
                    ap=[[LSP, BPC], [WROW, 4], [1, W]],
                )
                src = AP(
                    tensor=pst, offset=0, ap=[[512, BPC], [W, 4], [1, W]]
                )
                ecopy(k, dst, src)

            def emit_band(r0, r1, engines):
                # P9 partitions m=1..8 <- shifted copies of partitions {0,1}
                ln = (r1 - r0) * WROW
                n = 0
                for m in range(1, 9):
                    ip, jj = divmod(m, 3)
                    eng = engines[n % len(engines)]
                    n += 1
                    eng.dma_start(
                        out=AP(
                            tensor=p9t,
                            offset=2 * m * LSP + r0 * WROW,
                            ap=[[LSP, BPC], [1, ln]],
                        ),
                        in_=AP(
                            tensor=p9t,
                            offset=(r0 + jj) * WROW + ip,
                            ap=[[LSP, BPC], [1, ln]],
                        ),
                        single_packet=(eng is nc.gpsimd),
                    )

            out_engines = [nc.gpsimd, nc.sync]


            def emit_conv(j):
                cv = cvb[j % 4]
                nn = 512 if j < NCV - 1 else 256
                nc.tensor.matmul(
                    cv[:, :nn],
                    wb_t[:, :],
                    p9[:, 512 * j : 512 * j + nn],
                    start=True,
                    stop=True,
                )
                ecopy(j, yt[:, 512 * j : 512 * j + nn], cv[:, :nn])

            def emit_out(q):
                out_engines[q % 2].dma_start(
                    out=y.ap()[:, 2080 * q : 2080 * (q + 1)],
                    in_=yt[:, 2080 * q : 2080 * (q + 1)],
                )

            # out piece q (rows 16q..) ready after conv chunk: {chunk: piece}
            out_after = {4: 0, 8: 1, 12: 2, 16: 3, 20: 4, 24: 5, 28: 6, 32: 7}

            def emit_conv_full(j):
                emit_conv(j)
                if j in out_after:
                    emit_out(out_after[j])

            for k in range(32):
                emit_cs(k)
                if k == 12:
                    # band A (P9 flat [0, 6240)) needs staging rows <= 48
                    emit_band(0, 48, [nc.gpsimd])
            emit_band(48, H, [nc.gpsimd, nc.gpsimd, nc.sync])
            # dep-free warm matmuls: keep the PE busy through the band
            # wait so the HAM clock stays up for the conv phase
            for wkt in range(10):
                wps = csb[wkt % 4]
                nc.tensor.matmul(
                    wps[:, :],
                    ones_t[:, :],
                    xin[:, 0:512],
                    start=True,
                    stop=True,
                )
            for j in range(NCV):
                emit_conv_full(j)
            if dump:
                nc.sync.dma_start(out=p9_d.ap()[:, :], in_=p9[:, 0:HHW])

    nc.compile()
    return nc


def _host_prep(x, weight, bias):
    wsum = weight.sum(axis=1)  # [COUT, 3, 3]
    wb = np.zeros((NK, NOUT), np.float32)
    for b in range(BPC):
        for ip in range(3):
            for jj in range(3):
                wb[2 * (3 * ip + jj) + b, b * COUT : (b + 1) * COUT] = wsum[
                    :, 2 - jj, 2 - ip
                ]
    wb[NK - 1, :] = np.tile(bias, BPC)
    wb = wb.astype(np.float16)
    ones_p = np.ones((1, HHW), np.float16)

    in_maps = []
    for r in range(N_CORES):
        xhr = np.ascontiguousarray(
            x[r * BPC : (r + 1) * BPC].reshape(NPART, HW)
        ).astype(np.float16)
        in_maps.append({"xh": xhr, "wb": wb, "ones_p": ones_p})
    return in_maps


def kernel(x, weight, bias):
    from concourse.bass_utils import run_bass_kernel_spmd

    x = np.asarray(x)
    weight = np.asarray(weight)
    bias = np.asarray(bias)
    nc = _build()
    in_maps = _host_prep(x, weight, bias)
    res = run_bass_kernel_spmd(nc, in_maps, core_ids=list(range(N_CORES)))
    out = np.concatenate(
        [
            np.asarray(res.results[r]["y"])
            .astype(np.float32)
            .reshape(BPC, COUT, H, WROW)[:, :, :, :W]
            for r in range(N_CORES)
        ],
        axis=0,
    )
    return out


# revision 20
# speedup vs baseline: 1.0552x; 1.0151x over previous
"""FFTConv2d kernel for trn2, 8 NeuronCores.

Math: reference einsum 'bchw,oihw->bohw' factorizes:
  Y[b,o] = conv_same(sum_c x[b,c], flip(sum_i w[o,i])) + bias[o]
i.e. a single-channel 3x3 "same" convolution per (b,o) pair.

Per core (2 batches), all SBUF data fp16 (PSUM accum fp32):
  1. xin [128 (b,c), 16384] <- x fp16, 9 HBM DMA pieces (SP ring).
  2. Channel-sum: ones-indicator matmul pairs -> PSUM [2, 2x512];
     one FD=1024 copy (DVE/Act alternating) per pair drains 8 image
     rows into the padded staging = P9 partitions {0,1} (row stride
     130, zero borders memset once).
  3. P9 [19, 16902], partition 2m+b holds staging shifted by
     jj*130+i' (m=3i'+jj); m=0 IS the staging; m=1..8 built by 2-D
     self-copy DMAs in 2 row-bands (8 DMAs each, Pool/SP).
     Partition 18 = ones (bias rides the conv matmul).
  4. Conv: 33 flat 512-col chunks (yt rows are 130 wide, 2 junk cols
     stripped on host); K=19 fp16 matmul pairs into the same 4-deep
     2-bank PSUM rotation; FD=1024 contiguous copies -> yt.
  5. yt [128, 16640] -> HBM in 8 pieces on the Pool (SWDGE) ring,
     fp16; host strips junk cols + upconverts to fp32.

PSUM: four [128, 1024] fp32 tensors (2 banks each = all 8 banks),
time-shared: cs pairs use partitions 0:2, conv pairs all 128.
"""

import os
import sys
from functools import lru_cache

import numpy as np

for _p in ("/opt/trn_rl_repo", "/root/.axon_site/_ro/trn_rl_repo"):
    if os.path.isdir(_p) and _p not in sys.path:
        sys.path.insert(0, _p)

B, CIN, COUT, H, W = 16, 64, 64, 128, 128
N_CORES = 8
BPC = B // N_CORES  # 2
NPART = BPC * CIN  # 128
NOUT = BPC * COUT  # 128
WROW = W + 2  # 130
HW = H * W  # 16384
HHW = H * WROW  # 16640 (130-wide output rows)
LSP = (H + 2) * WROW + 2  # 16902 (padded staging length)
NK = BPC * 9 + 1  # 19
NCV = 33  # conv chunks: 32x512 + 1x256


@lru_cache(maxsize=1)
def _build():
    import concourse.bacc as bacc
    import concourse.mybir as mybir
    import concourse.tile as tile
    from concourse.ap import AP

    f32 = mybir.dt.float32
    f16 = mybir.dt.float16

    nc = bacc.Bacc("TRN2", target_bir_lowering=False, debug=False, num_devices=N_CORES)

    xh = nc.dram_tensor("xh", [NPART, HW], f16, kind="ExternalInput")
    wbh = nc.dram_tensor("wb", [NK, NOUT], f16, kind="ExternalInput")
    onesp = nc.dram_tensor("ones_p", [1, HHW], f16, kind="ExternalInput")
    y = nc.dram_tensor("y", [NOUT, HHW], f16, kind="ExternalOutput")
    dump = os.environ.get("KDUMP")
    if dump:
        p9_d = nc.dram_tensor("p9_d", [NK, HHW], f16, kind="ExternalOutput")

    with tile.TileContext(nc) as tc:
        with (
            tc.tile_pool(name="main", bufs=1) as mp,
            tc.tile_pool(name="ps", bufs=1, space="PSUM") as ps_pool,
        ):
            xin = mp.tile([NPART, HW], f16, tag="xin")
            p9 = mp.tile([NK, LSP], f16, tag="p9")
            yt = mp.tile([NOUT, HHW], f16, tag="yt")
            ones_t = mp.tile([NPART, BPC], f16, tag="ones_t")
            wb_t = mp.tile([NK, NOUT], f16, tag="wb")

            p9t = p9.tensor

            csb = [
                ps_pool.tile([BPC, 512], f32, tag=f"cs{i}", name=f"cs{i}")
                for i in range(4)
            ]
            cvb = [
                ps_pool.tile([NOUT, 512], f32, tag=f"cv{i}", name=f"cv{i}")
                for i in range(4)
            ]

            # constants on the idle Pool (SWDGE) ring so the Act queue
            # starts clean (a late Act start skews the static schedule)
            nc.gpsimd.dma_start(out=wb_t[:, :], in_=wbh.ap()[:, :])
            nc.gpsimd.dma_start(out=p9[NK - 1 : NK, 0:HHW], in_=onesp.ap()[0:1, :])

            # ones indicator [128, 2]: col b is 1 for partitions of batch b
            nc.vector.memset(ones_t[0:CIN, 0:1], 1.0)
            nc.vector.memset(ones_t[0:CIN, 1:2], 0.0)
            nc.vector.memset(ones_t[CIN:NPART, 0:1], 0.0)
            nc.vector.memset(ones_t[CIN:NPART, 1:2], 1.0)

            # staging zero borders in P9 partitions {0, 1}:
            # row -1, row 128 + tail, and (right col, next left col) pairs
            nc.vector.memset(
                AP(tensor=p9t, offset=0, ap=[[LSP, BPC], [1, WROW]]), 0.0
            )
            nc.vector.memset(
                AP(
                    tensor=p9t,
                    offset=(H + 1) * WROW,
                    ap=[[LSP, BPC], [1, LSP - (H + 1) * WROW]],
                ),
                0.0,
            )
            nc.vector.memset(
                AP(
                    tensor=p9t,
                    offset=WROW - 1,
                    ap=[[LSP, BPC], [WROW, H + 1], [1, 2]],
                ),
                0.0,
            )

            # input: 9 pieces on the SP ring
            pieces = [(0, 1024), (1024, 1024)] + [
                (2048 * q, 2048) for q in range(1, 8)
            ]
            for o, n in pieces:
                nc.sync.dma_start(out=xin[:, o : o + n], in_=xh.ap()[:, o : o + n])

            copy_engines = [nc.vector, nc.scalar]

            def ecopy(idx, dst, src):
                eng = copy_engines[idx % 2]
                if eng is nc.vector:
                    eng.tensor_copy(dst, src)
                else:
                    eng.copy(dst, src)

            def emit_cs(k):
                # ones-matmul of 512 cols (4 rows) -> [2, 512]; 1 copy
                ps = csb[k % 4]
                pst = ps.tensor
                nc.tensor.matmul(
                    ps[:, :],
                    ones_t[:, :],
                    xin[:, 512 * k : 512 * k + 512],
                    start=True,
                    stop=True,
                )
                dst = AP(
                    tensor=p9t,
                    offset=(4 * k + 1) * WROW + 1,
                    ap=[[LSP, BPC], [WROW, 4], [1, W]],
                )
                src = AP(
                    tensor=pst, offset=0, ap=[[512, BPC], [W, 4], [1, W]]
                )
                ecopy(k, dst, src)

            def emit_band(r0, r1, engines):
                # P9 partitions m=1..8 <- shifted copies of partitions {0,1}
                ln = (r1 - r0) * WROW
                n = 0
                for m in range(1, 9):
                    ip, jj = divmod(m, 3)
                    eng = engines[n % len(engines)]
                    n += 1
                    eng.dma_start(
                        out=AP(
                            tensor=p9t,
                            offset=2 * m * LSP + r0 * WROW,
                            ap=[[LSP, BPC], [1, ln]],
                        ),
                        in_=AP(
                            tensor=p9t,
                            offset=(r0 + jj) * WROW + ip,
                            ap=[[LSP, BPC], [1, ln]],
                        ),
                        single_packet=(eng is nc.gpsimd),
                    )

            out_engines = [nc.gpsimd, nc.sync]


            def emit_conv(j):
                cv = cvb[j % 4]
                nn = 512 if j < NCV - 1 else 256
                nc.tensor.matmul(
                    cv[:, :nn],
                    wb_t[:, :],
                    p9[:, 512 * j : 512 * j + nn],
                    start=True,
                    stop=True,
                )
                ecopy(j, yt[:, 512 * j : 512 * j + nn], cv[:, :nn])

            def emit_out(q):
                out_engines[q % 2].dma_start(
                    out=y.ap()[:, 2080 * q : 2080 * (q + 1)],
                    in_=yt[:, 2080 * q : 2080 * (q + 1)],
                )

            # out piece q (rows 16q..) ready after conv chunk: {chunk: piece}
            out_after = {4: 0, 8: 1, 12: 2, 16: 3, 20: 4, 24: 5, 28: 6, 32: 7}

            def emit_conv_full(j):
                emit_conv(j)
                if j in out_after:
                    emit_out(out_after[j])

            for k in range(32):
                emit_cs(k)
                if k == 12:
                    # band A (P9 flat [0, 6240)) needs staging rows <= 48
                    emit_band(0, 48, [nc.gpsimd])
            emit_band(48, H, [nc.gpsimd, nc.gpsimd, nc.sync])
            for j in range(NCV):
                emit_conv_full(j)
            if dump:
                nc.sync.dma_start(out=p9_d.ap()[:, :], in_=p9[:, 0:HHW])

    nc.compile()
    return nc


def _host_prep(x, weight, bias):
    wsum = weight.sum(axis=1)  # [COUT, 3, 3]
    wb = np.zeros((NK, NOUT), np.float32)
    for b in range(BPC):
        for ip in range(3):
            for jj in range(3):
                wb[2 * (3 * ip + jj) + b, b * COUT : (b + 1) * COUT] = wsum[
                    :, 2 - jj, 2 - ip
                ]
    wb[NK - 1, :] = np.tile(bias, BPC)
    wb = wb.astype(np.float16)
    ones_p = np.ones((1, HHW), np.float16)

    in_maps = []
    for r in range(N_CORES):
        xhr = np.ascontiguousarray(
            x[r * BPC : (r + 1) * BPC].reshape(NPART, HW)
        ).astype(np.float16)
        in_maps.append({"xh": xhr, "wb": wb, "ones_p": ones_p})
    return in_maps


def kernel(x, weight, bias):
    from concourse.bass_utils import run_bass_kernel_spmd

    x = np.asarray(x)
    weight = np.asarray(weight)
    bias = np.asarray(bias)
    nc = _build()
    in_maps = _host_prep(x, weight, bias)
    res = run_bass_kernel_spmd(nc, in_maps, core_ids=list(range(N_CORES)))
    out = np.concatenate(
        [
            np.asarray(res.results[r]["y"])
            .astype(np.float32)
            .reshape(BPC, COUT, H, WROW)[:, :, :, :W]
            for r in range(N_CORES)
        ],
        axis=0,
    )
    return out


# revision 21
# speedup vs baseline: 1.0676x; 1.0118x over previous
"""FFTConv2d kernel for trn2, 8 NeuronCores.

Math: reference einsum 'bchw,oihw->bohw' factorizes:
  Y[b,o] = conv_same(sum_c x[b,c], flip(sum_i w[o,i])) + bias[o]
i.e. a single-channel 3x3 "same" convolution per (b,o) pair.

Per core (2 batches), all SBUF data fp16 (PSUM accum fp32):
  1. xin [128 (b,c), 16384] <- x fp16, 9 HBM DMA pieces (SP ring).
  2. Channel-sum: ones-indicator matmul pairs -> PSUM [2, 2x512];
     one FD=1024 copy (DVE/Act alternating) per pair drains 8 image
     rows into the padded staging = P9 partitions {0,1} (row stride
     130, zero borders memset once).
  3. P9 [19, 16902], partition 2m+b holds staging shifted by
     jj*130+i' (m=3i'+jj); m=0 IS the staging; m=1..8 built by 2-D
     self-copy DMAs in 2 row-bands (8 DMAs each, Pool/SP).
     Partition 18 = ones (bias rides the conv matmul).
  4. Conv: 33 flat 512-col chunks (yt rows are 130 wide, 2 junk cols
     stripped on host); K=19 fp16 matmul pairs into the same 4-deep
     2-bank PSUM rotation; FD=1024 contiguous copies -> yt.
  5. yt [128, 16640] -> HBM in 8 pieces on the Pool (SWDGE) ring,
     fp16; host strips junk cols + upconverts to fp32.

PSUM: four [128, 1024] fp32 tensors (2 banks each = all 8 banks),
time-shared: cs pairs use partitions 0:2, conv pairs all 128.
"""

import os
import sys
from functools import lru_cache

import numpy as np

for _p in ("/opt/trn_rl_repo", "/root/.axon_site/_ro/trn_rl_repo"):
    if os.path.isdir(_p) and _p not in sys.path:
        sys.path.insert(0, _p)

B, CIN, COUT, H, W = 16, 64, 64, 128, 128
N_CORES = 8
BPC = B // N_CORES  # 2
NPART = BPC * CIN  # 128
NOUT = BPC * COUT  # 128
WROW = W + 2  # 130
HW = H * W  # 16384
HHW = H * WROW  # 16640 (130-wide output rows)
LSP = (H + 2) * WROW + 2  # 16902 (padded staging length)
NK = BPC * 9 + 1  # 19
NCV = 33  # conv chunks: 32x512 + 1x256


@lru_cache(maxsize=1)
def _build():
    import concourse.bacc as bacc
    import concourse.mybir as mybir
    import concourse.tile as tile
    from concourse.ap import AP

    f32 = mybir.dt.float32
    f16 = mybir.dt.float16

    nc = bacc.Bacc("TRN2", target_bir_lowering=False, debug=False, num_devices=N_CORES)

    xh = nc.dram_tensor("xh", [NPART, HW], f16, kind="ExternalInput")
    wbh = nc.dram_tensor("wb", [NK, NOUT], f16, kind="ExternalInput")
    onesp = nc.dram_tensor("ones_p", [1, HHW], f16, kind="ExternalInput")
    y = nc.dram_tensor("y", [NOUT, HHW], f16, kind="ExternalOutput")
    dump = os.environ.get("KDUMP")
    if dump:
        p9_d = nc.dram_tensor("p9_d", [NK, HHW], f16, kind="ExternalOutput")

    with tile.TileContext(nc) as tc:
        with (
            tc.tile_pool(name="main", bufs=1) as mp,
            tc.tile_pool(name="ps", bufs=1, space="PSUM") as ps_pool,
        ):
            xin = mp.tile([NPART, HW], f16, tag="xin")
            p9 = mp.tile([NK, LSP], f16, tag="p9")
            yt = mp.tile([NOUT, HHW], f16, tag="yt")
            ones_t = mp.tile([NPART, BPC], f16, tag="ones_t")
            wb_t = mp.tile([NK, NOUT], f16, tag="wb")

            p9t = p9.tensor

            csb = [
                ps_pool.tile([BPC, 512], f32, tag=f"cs{i}", name=f"cs{i}")
                for i in range(4)
            ]
            cvb = [
                ps_pool.tile([NOUT, 512], f32, tag=f"cv{i}", name=f"cv{i}")
                for i in range(4)
            ]

            # constants on the idle Pool (SWDGE) ring so the Act queue
            # starts clean (a late Act start skews the static schedule)
            nc.gpsimd.dma_start(out=wb_t[:, :], in_=wbh.ap()[:, :])
            nc.gpsimd.dma_start(out=p9[NK - 1 : NK, 0:HHW], in_=onesp.ap()[0:1, :])

            # ones indicator [128, 2]: col b is 1 for partitions of batch b
            nc.vector.memset(ones_t[0:CIN, 0:1], 1.0)
            nc.vector.memset(ones_t[0:CIN, 1:2], 0.0)
            nc.vector.memset(ones_t[CIN:NPART, 0:1], 0.0)
            nc.vector.memset(ones_t[CIN:NPART, 1:2], 1.0)

            # staging zero borders in P9 partitions {0, 1}:
            # row -1, row 128 + tail, and (right col, next left col) pairs
            nc.vector.memset(
                AP(tensor=p9t, offset=0, ap=[[LSP, BPC], [1, WROW]]), 0.0
            )
            nc.vector.memset(
                AP(
                    tensor=p9t,
                    offset=(H + 1) * WROW,
                    ap=[[LSP, BPC], [1, LSP - (H + 1) * WROW]],
                ),
                0.0,
            )
            nc.vector.memset(
                AP(
                    tensor=p9t,
                    offset=WROW - 1,
                    ap=[[LSP, BPC], [WROW, H + 1], [1, 2]],
                ),
                0.0,
            )

            # input: 9 pieces on the SP ring
            pieces = [(0, 1024), (1024, 1024)] + [
                (2048 * q, 2048) for q in range(1, 8)
            ]
            for o, n in pieces:
                nc.sync.dma_start(out=xin[:, o : o + n], in_=xh.ap()[:, o : o + n])

            copy_engines = [nc.vector, nc.scalar]

            def ecopy(idx, dst, src):
                eng = copy_engines[idx % 2]
                if eng is nc.vector:
                    eng.tensor_copy(dst, src)
                else:
                    eng.copy(dst, src)

            def emit_cs(k):
                # ones-matmul of 512 cols (4 rows) -> [2, 512]; 1 copy
                ps = csb[k % 4]
                pst = ps.tensor
                nc.tensor.matmul(
                    ps[:, :],
                    ones_t[:, :],
                    xin[:, 512 * k : 512 * k + 512],
                    start=True,
                    stop=True,
                )
                dst = AP(
                    tensor=p9t,
                    offset=(4 * k + 1) * WROW + 1,
                    ap=[[LSP, BPC], [WROW, 4], [1, W]],
                )
                src = AP(
                    tensor=pst, offset=0, ap=[[512, BPC], [W, 4], [1, W]]
                )
                ecopy(k, dst, src)

            def emit_band(r0, r1, engines):
                # P9 partitions m=1..8 <- shifted copies of partitions {0,1}
                ln = (r1 - r0) * WROW
                n = 0
                for m in range(1, 9):
                    ip, jj = divmod(m, 3)
                    eng = engines[n % len(engines)]
                    n += 1
                    eng.dma_start(
                        out=AP(
                            tensor=p9t,
                            offset=2 * m * LSP + r0 * WROW,
                            ap=[[LSP, BPC], [1, ln]],
                        ),
                        in_=AP(
                            tensor=p9t,
                            offset=(r0 + jj) * WROW + ip,
                            ap=[[LSP, BPC], [1, ln]],
                        ),
                    )

            out_engines = [nc.gpsimd, nc.sync]


            def emit_conv(j):
                cv = cvb[j % 4]
                nn = 512 if j < NCV - 1 else 256
                nc.tensor.matmul(
                    cv[:, :nn],
                    wb_t[:, :],
                    p9[:, 512 * j : 512 * j + nn],
                    start=True,
                    stop=True,
                )
                ecopy(j, yt[:, 512 * j : 512 * j + nn], cv[:, :nn])

            def emit_out(q):
                out_engines[q % 2].dma_start(
                    out=y.ap()[:, 2080 * q : 2080 * (q + 1)],
                    in_=yt[:, 2080 * q : 2080 * (q + 1)],
                )

            # out piece q (rows 16q..) ready after conv chunk: {chunk: piece}
            out_after = {4: 0, 8: 1, 12: 2, 16: 3, 20: 4, 24: 5, 28: 6, 32: 7}

            def emit_conv_full(j):
                emit_conv(j)
                if j in out_after:
                    emit_out(out_after[j])

            for k in range(32):
                emit_cs(k)
                if k == 12:
                    # band A (P9 flat [0, 6240)) needs staging rows <= 48
                    emit_band(0, 48, [nc.gpsimd])
            emit_band(48, H, [nc.gpsimd, nc.gpsimd, nc.sync])
            for j in range(NCV):
                emit_conv_full(j)
            if dump:
                nc.sync.dma_start(out=p9_d.ap()[:, :], in_=p9[:, 0:HHW])

    nc.compile()
    return nc


def _host_prep(x, weight, bias):
    wsum = weight.sum(axis=1)  # [COUT, 3, 3]
    wb = np.zeros((NK, NOUT), np.float32)
    for b in range(BPC):
        for ip in range(3):
            for jj in range(3):
                wb[2 * (3 * ip + jj) + b, b * COUT : (b + 1) * COUT] = wsum[
                    :, 2 - jj, 2 - ip
                ]
    wb[NK - 1, :] = np.tile(bias, BPC)
    wb = wb.astype(np.float16)
    ones_p = np.ones((1, HHW), np.float16)

    in_maps = []
    for r in range(N_CORES):
        xhr = np.ascontiguousarray(
            x[r * BPC : (r + 1) * BPC].reshape(NPART, HW)
        ).astype(np.float16)
        in_maps.append({"xh": xhr, "wb": wb, "ones_p": ones_p})
    return in_maps


def kernel(x, weight, bias):
    from concourse.bass_utils import run_bass_kernel_spmd

    x = np.asarray(x)
    weight = np.asarray(weight)
    bias = np.asarray(bias)
    nc = _build()
    in_maps = _host_prep(x, weight, bias)
    res = run_bass_kernel_spmd(nc, in_maps, core_ids=list(range(N_CORES)))
    out = np.concatenate(
        [
            np.asarray(res.results[r]["y"])
            .astype(np.float32)
            .reshape(BPC, COUT, H, WROW)[:, :, :, :W]
            for r in range(N_CORES)
        ],
        axis=0,
    )
    return out


# revision 22
# speedup vs baseline: 1.0967x; 1.0273x over previous
"""FFTConv2d kernel for trn2, 8 NeuronCores.

Math: reference einsum 'bchw,oihw->bohw' factorizes:
  Y[b,o] = conv_same(sum_c x[b,c], flip(sum_i w[o,i])) + bias[o]
i.e. a single-channel 3x3 "same" convolution per (b,o) pair.

Per core (2 batches), all SBUF data fp16 (PSUM accum fp32):
  1. xin [128 (b,c), 16384] <- x fp16, 9 HBM DMA pieces (SP ring).
  2. Channel-sum: ones-indicator matmul pairs -> PSUM [2, 2x512];
     one FD=1024 copy (DVE/Act alternating) per pair drains 8 image
     rows into the padded staging = P9 partitions {0,1} (row stride
     130, zero borders memset once).
  3. P9 [19, 16902], partition 2m+b holds staging shifted by
     jj*130+i' (m=3i'+jj); m=0 IS the staging; m=1..8 built by 2-D
     self-copy DMAs in 2 row-bands (8 DMAs each, Pool/SP).
     Partition 18 = ones (bias rides the conv matmul).
  4. Conv: 33 flat 512-col chunks (yt rows are 130 wide, 2 junk cols
     stripped on host); K=19 fp16 matmul pairs into the same 4-deep
     2-bank PSUM rotation; FD=1024 contiguous copies -> yt.
  5. yt [128, 16640] -> HBM in 8 pieces on the Pool (SWDGE) ring,
     fp16; host strips junk cols + upconverts to fp32.

PSUM: four [128, 1024] fp32 tensors (2 banks each = all 8 banks),
time-shared: cs pairs use partitions 0:2, conv pairs all 128.
"""

import os
import sys
from functools import lru_cache

import numpy as np

for _p in ("/opt/trn_rl_repo", "/root/.axon_site/_ro/trn_rl_repo"):
    if os.path.isdir(_p) and _p not in sys.path:
        sys.path.insert(0, _p)

B, CIN, COUT, H, W = 16, 64, 64, 128, 128
N_CORES = 8
BPC = B // N_CORES  # 2
NPART = BPC * CIN  # 128
NOUT = BPC * COUT  # 128
WROW = W + 2  # 130
HW = H * W  # 16384
HHW = H * WROW  # 16640 (130-wide output rows)
LSP = (H + 2) * WROW + 2  # 16902 (padded staging length)
NK = BPC * 9 + 1  # 19
NCV = 33  # conv chunks: 32x512 + 1x256


@lru_cache(maxsize=1)
def _build():
    import concourse.bacc as bacc
    import concourse.mybir as mybir
    import concourse.tile as tile
    from concourse.ap import AP

    f32 = mybir.dt.float32
    f16 = mybir.dt.float16

    nc = bacc.Bacc("TRN2", target_bir_lowering=False, debug=False, num_devices=N_CORES)

    xh = nc.dram_tensor("xh", [NPART, HW], f16, kind="ExternalInput")
    wbh = nc.dram_tensor("wb", [NK, NOUT], f16, kind="ExternalInput")
    onesp = nc.dram_tensor("ones_p", [1, HHW], f16, kind="ExternalInput")
    y = nc.dram_tensor("y", [NOUT, HHW], f16, kind="ExternalOutput")
    dump = os.environ.get("KDUMP")
    if dump:
        p9_d = nc.dram_tensor("p9_d", [NK, HHW], f16, kind="ExternalOutput")

    with tile.TileContext(nc) as tc:
        with (
            tc.tile_pool(name="main", bufs=1) as mp,
            tc.tile_pool(name="ps", bufs=1, space="PSUM") as ps_pool,
        ):
            xin = mp.tile([NPART, HW], f16, tag="xin")
            p9 = mp.tile([NK, LSP], f16, tag="p9")
            yt = mp.tile([NOUT, HHW], f16, tag="yt")
            ones_t = mp.tile([NPART, BPC], f16, tag="ones_t")
            wb_t = mp.tile([NK, NOUT], f16, tag="wb")

            p9t = p9.tensor

            csb = [
                ps_pool.tile([BPC, 512], f32, tag=f"cs{i}", name=f"cs{i}")
                for i in range(4)
            ]
            cvb = [
                ps_pool.tile([NOUT, 512], f32, tag=f"cv{i}", name=f"cv{i}")
                for i in range(4)
            ]

            # constants on the idle Pool (SWDGE) ring so the Act queue
            # starts clean (a late Act start skews the static schedule)
            nc.gpsimd.dma_start(out=wb_t[:, :], in_=wbh.ap()[:, :])
            nc.gpsimd.dma_start(out=p9[NK - 1 : NK, 0:HHW], in_=onesp.ap()[0:1, :])

            # ones indicator [128, 2]: col b is 1 for partitions of batch b
            nc.vector.memset(ones_t[0:CIN, 0:1], 1.0)
            nc.vector.memset(ones_t[0:CIN, 1:2], 0.0)
            nc.vector.memset(ones_t[CIN:NPART, 0:1], 0.0)
            nc.vector.memset(ones_t[CIN:NPART, 1:2], 1.0)

            # staging zero borders in P9 partitions {0, 1}:
            # row -1, row 128 + tail, and (right col, next left col) pairs
            nc.vector.memset(
                AP(tensor=p9t, offset=0, ap=[[LSP, BPC], [1, WROW]]), 0.0
            )
            nc.vector.memset(
                AP(
                    tensor=p9t,
                    offset=(H + 1) * WROW,
                    ap=[[LSP, BPC], [1, LSP - (H + 1) * WROW]],
                ),
                0.0,
            )
            nc.vector.memset(
                AP(
                    tensor=p9t,
                    offset=WROW - 1,
                    ap=[[LSP, BPC], [WROW, H + 1], [1, 2]],
                ),
                0.0,
            )

            # input: 9 pieces on the SP ring
            pieces = [(0, 1024), (1024, 1024)] + [
                (2048 * q, 2048) for q in range(1, 8)
            ]
            for o, n in pieces:
                nc.sync.dma_start(out=xin[:, o : o + n], in_=xh.ap()[:, o : o + n])

            copy_engines = [nc.vector, nc.scalar]

            def ecopy(idx, dst, src):
                eng = copy_engines[idx % 2]
                if eng is nc.vector:
                    eng.tensor_copy(dst, src)
                else:
                    eng.copy(dst, src)

            def emit_cs(k):
                # ones-matmul of 512 cols (4 rows) -> [2, 512]; 1 copy
                ps = csb[k % 4]
                pst = ps.tensor
                nc.tensor.matmul(
                    ps[:, :],
                    ones_t[:, :],
                    xin[:, 512 * k : 512 * k + 512],
                    start=True,
                    stop=True,
                )
                dst = AP(
                    tensor=p9t,
                    offset=(4 * k + 1) * WROW + 1,
                    ap=[[LSP, BPC], [WROW, 4], [1, W]],
                )
                src = AP(
                    tensor=pst, offset=0, ap=[[512, BPC], [W, 4], [1, W]]
                )
                ecopy(k, dst, src)

            def emit_band(r0, r1, engines):
                # P9 partitions m=1..8 <- shifted copies of partitions {0,1}
                ln = (r1 - r0) * WROW
                n = 0
                for m in range(1, 9):
                    ip, jj = divmod(m, 3)
                    eng = engines[n % len(engines)]
                    n += 1
                    eng.dma_start(
                        out=AP(
                            tensor=p9t,
                            offset=2 * m * LSP + r0 * WROW,
                            ap=[[LSP, BPC], [1, ln]],
                        ),
                        in_=AP(
                            tensor=p9t,
                            offset=(r0 + jj) * WROW + ip,
                            ap=[[LSP, BPC], [1, ln]],
                        ),
                    )

            out_engines = [nc.gpsimd, nc.sync]


            def emit_conv(j):
                cv = cvb[j % 4]
                nn = 512 if j < NCV - 1 else 256
                nc.tensor.matmul(
                    cv[:, :nn],
                    wb_t[:, :],
                    p9[:, 512 * j : 512 * j + nn],
                    start=True,
                    stop=True,
                )
                ecopy(j, yt[:, 512 * j : 512 * j + nn], cv[:, :nn])

            def emit_out(q):
                out_engines[q % 2].dma_start(
                    out=y.ap()[:, 2080 * q : 2080 * (q + 1)],
                    in_=yt[:, 2080 * q : 2080 * (q + 1)],
                )

            # out piece q (rows 16q..) ready after conv chunk: {chunk: piece}
            out_after = {4: 0, 8: 1, 12: 2, 16: 3, 20: 4, 24: 5, 28: 6, 32: 7}

            def emit_conv_full(j):
                emit_conv(j)
                if j in out_after:
                    emit_out(out_after[j])

            for k in range(32):
                emit_cs(k)
                if k == 12:
                    # band A (P9 flat [0, 6240)) needs staging rows <= 48
                    emit_band(0, 48, [nc.gpsimd])
                if k == 24:
                    # band B1 (rows 48..95) needs staging rows <= 96
                    emit_band(48, 96, [nc.gpsimd, nc.sync])
            emit_band(96, H, [nc.gpsimd, nc.sync])
            for j in range(NCV):
                emit_conv_full(j)
            if dump:
                nc.sync.dma_start(out=p9_d.ap()[:, :], in_=p9[:, 0:HHW])

    nc.compile()
    return nc


def _host_prep(x, weight, bias):
    wsum = weight.sum(axis=1)  # [COUT, 3, 3]
    wb = np.zeros((NK, NOUT), np.float32)
    for b in range(BPC):
        for ip in range(3):
            for jj in range(3):
                wb[2 * (3 * ip + jj) + b, b * COUT : (b + 1) * COUT] = wsum[
                    :, 2 - jj, 2 - ip
                ]
    wb[NK - 1, :] = np.tile(bias, BPC)
    wb = wb.astype(np.float16)
    ones_p = np.ones((1, HHW), np.float16)

    in_maps = []
    for r in range(N_CORES):
        xhr = np.ascontiguousarray(
            x[r * BPC : (r + 1) * BPC].reshape(NPART, HW)
        ).astype(np.float16)
        in_maps.append({"xh": xhr, "wb": wb, "ones_p": ones_p})
    return in_maps


def kernel(x, weight, bias):
    from concourse.bass_utils import run_bass_kernel_spmd

    x = np.asarray(x)
    weight = np.asarray(weight)
    bias = np.asarray(bias)
    nc = _build()
    in_maps = _host_prep(x, weight, bias)
    res = run_bass_kernel_spmd(nc, in_maps, core_ids=list(range(N_CORES)))
    out = np.concatenate(
        [
            np.asarray(res.results[r]["y"])
            .astype(np.float32)
            .reshape(BPC, COUT, H, WROW)[:, :, :, :W]
            for r in range(N_CORES)
        ],
        axis=0,
    )
    return out
